# revision 17
# baseline (speedup 1.0000x reference)
"""Causal self-attention (B=2, T=2048, E=1024, H=16, d_k=64) on 8 TRN2 cores.

Tensor-parallel over heads: core c owns heads 2c, 2c+1 (feature slice
c*128:(c+1)*128 of the QKV projections and of the Wo contraction dim).
Each core computes a partial output [4096, 1024]; the host sums the 8
partials and adds bo.
"""

import numpy as np

B = 2
T = 2048
E = 1024
F = 128          # per-core QKV features (2 heads x 64)
DK = 64
NH_LOC = 2       # heads per core
N_CORES = 8
TT = B * T       # flattened tokens
IC = 512         # query chunk (moving free dim)
JC = 128         # key chunk (stationary free dim)
NEG = -1.0e9     # pre-scale mask value; exp(0.125 * NEG) == 0 in fp32

_CACHE = {}


def _build_program(tokens_per_batch, n_batch, embd, debug_taps=False):
    import concourse.mybir as mybir
    import concourse.tile as tile
    from concourse import bacc

    f32 = mybir.dt.float32
    Act = mybir.ActivationFunctionType

    t_total = tokens_per_batch * n_batch        # all tokens
    n_ec = embd // 128                          # e-chunks (contraction)
    n_tc5 = t_total // IC                       # 512-token chunks
    n_tc = t_total // JC                        # 128-token chunks
    n_ic = tokens_per_batch // IC               # query chunks per batch
    fc_w = min(IC, embd)                        # output projection chunk width
    n_fc = embd // fc_w                         # output feature chunks

    nc = bacc.Bacc("TRN2", target_bir_lowering=False, debug=False)

    x_ap = nc.dram_tensor("x", [t_total, embd], f32, kind="ExternalInput").ap()
    wqT = nc.dram_tensor("wqT", [embd, F], f32, kind="ExternalInput").ap()
    wkT = nc.dram_tensor("wkT", [embd, F], f32, kind="ExternalInput").ap()
    wvT = nc.dram_tensor("wvT", [embd, F], f32, kind="ExternalInput").ap()
    woT = nc.dram_tensor("woT", [F, embd], f32, kind="ExternalInput").ap()
    bq_ap = nc.dram_tensor("bq", [F], f32, kind="ExternalInput").ap()
    bk_ap = nc.dram_tensor("bk", [F], f32, kind="ExternalInput").ap()
    bv_ap = nc.dram_tensor("bv", [F], f32, kind="ExternalInput").ap()
    masks_ap = nc.dram_tensor("masks", [4, JC, IC], f32, kind="ExternalInput").ap()
    ident_ap = nc.dram_tensor("ident", [128, 128], f32, kind="ExternalInput").ap()
    out_ap = nc.dram_tensor("partial", [t_total, embd], f32, kind="ExternalOutput").ap()
    if debug_taps:
        n_tc_dbg = t_total // JC
        dbg_qt = nc.dram_tensor("dbg_qt", [128, t_total], f32, kind="ExternalOutput").ap()
        dbg_kt = nc.dram_tensor("dbg_kt", [128, t_total], f32, kind="ExternalOutput").ap()
        dbg_v1 = nc.dram_tensor("dbg_v1", [128, n_tc_dbg, 132], f32, kind="ExternalOutput").ap()
        dbg_y = nc.dram_tensor("dbg_y", [128, n_tc_dbg, F], f32, kind="ExternalOutput").ap()
        dbg_xt = nc.dram_tensor("dbg_xt", [128, embd // 128, IC], f32, kind="ExternalOutput").ap()
        dbg_yt = nc.dram_tensor("dbg_yt", [128, 128], f32, kind="ExternalOutput").ap()
        dbg_ob = nc.dram_tensor("dbg_ob", [128, min(IC, embd)], f32, kind="ExternalOutput").ap()

    with tile.TileContext(nc) as tc:
        with (
            tc.tile_pool(name="const", bufs=1) as constp,
            tc.tile_pool(name="persist", bufs=1) as persist,
            tc.tile_pool(name="xload", bufs=3) as xload,
            tc.tile_pool(name="xt", bufs=2) as xtp,
            tc.tile_pool(name="pt", bufs=17) as ptp,
            tc.tile_pool(name="work", bufs=3) as work,
            tc.tile_pool(name="outs", bufs=3) as outsp,
            tc.tile_pool(name="ps_big", bufs=3, space="PSUM") as ps_big,
            tc.tile_pool(name="ps_small", bufs=3, space="PSUM") as ps_small,
            tc.tile_pool(name="ps_y", bufs=2, space="PSUM") as ps_y,
        ):
            # ---- constants ----
            wq_sb = constp.tile([128, n_ec, F], f32, tag="wq")
            nc.sync.dma_start(wq_sb[:], wqT.rearrange("(a p) f -> p a f", p=128))
            wk_sb = constp.tile([128, n_ec, F], f32, tag="wk")
            nc.sync.dma_start(wk_sb[:], wkT.rearrange("(a p) f -> p a f", p=128))
            wv_sb = constp.tile([128, n_ec, F], f32, tag="wv")
            nc.sync.dma_start(wv_sb[:], wvT.rearrange("(a p) f -> p a f", p=128))
            wo_sb = constp.tile([128, embd], f32, tag="wo")
            nc.sync.dma_start(wo_sb[:], woT)
            bq_sb = constp.tile([128, 1], f32, tag="bq")
            nc.sync.dma_start(bq_sb[:], bq_ap[:])
            bk_sb = constp.tile([128, 1], f32, tag="bk")
            nc.sync.dma_start(bk_sb[:], bk_ap[:])
            bv_sb = constp.tile([1, F], f32, tag="bv")
            nc.sync.dma_start(bv_sb[:], bv_ap[:])
            masks_sb = constp.tile([128, 4, IC], f32, tag="masks")
            nc.sync.dma_start(masks_sb[:], masks_ap.rearrange("o p i -> p o i"))
            ident = constp.tile([128, 128], f32, tag="ident")
            nc.sync.dma_start(ident[:], ident_ap)
            ones_sb = constp.tile([1, 128], f32, tag="ones")
            nc.vector.memset(ones_sb[:], 1.0)

            # ---- persistent activations ----
            qt_sb = persist.tile([128, t_total], f32, tag="qt")   # [f, t]
            kt_sb = persist.tile([128, t_total], f32, tag="kt")   # [f, t]
            # V plus a ones column per head: head h at cols h*66 .. h*66+64
            v1_sb = persist.tile([128, n_tc, 2 * 66], f32, tag="v1")
            y_sb = persist.tile([128, n_tc, F], f32, tag="y")     # [t, e'] per chunk

            for h in range(NH_LOC):
                nc.vector.memset(v1_sb[:, :, h * 66 + 64], 1.0)
                nc.vector.memset(v1_sb[:, :, h * 66 + 65], 0.0)

            # ---- phase B: transpose x, project to QT/KT/V ----
            for tc5 in range(n_tc5):
                t0 = tc5 * IC
                xt = xtp.tile([128, n_ec, IC], f32, tag="xt")  # [e, ec, t]
                for sub in range(IC // 128):
                    xrow = xload.tile([128, embd], f32, tag="xrow")
                    nc.sync.dma_start(
                        xrow[:], x_ap[t0 + sub * 128 : t0 + (sub + 1) * 128, :]
                    )
                    for ec in range(n_ec):
                        tp = ps_small.tile([128, 128], f32, tag="tp")
                        nc.tensor.transpose(
                            tp[:], xrow[:, ec * 128 : (ec + 1) * 128], ident[:]
                        )
                        nc.vector.tensor_copy(
                            xt[:, ec, sub * 128 : (sub + 1) * 128], tp[:]
                        )
                if debug_taps and tc5 == 0:
                    nc.sync.dma_start(dbg_xt[:], xt[:])
                # QT / KT chunks
                for dst, w_sb, b_sb in ((qt_sb, wq_sb, bq_sb), (kt_sb, wk_sb, bk_sb)):
                    ps = ps_big.tile([128, IC], f32, tag="psb")
                    for ec in range(n_ec):
                        nc.tensor.matmul(
                            ps[:],
                            w_sb[:, ec, :],
                            xt[:, ec, :],
                            start=(ec == 0),
                            stop=(ec == n_ec - 1),
                        )
                    nc.scalar.activation(
                        dst[:, t0 : t0 + IC], ps[:], Act.Identity, bias=b_sb[:]
                    )
                # V chunks (natural [t, f] layout), bias seeded via rank-1 matmul
                for sub in range(IC // 128):
                    vps = ps_small.tile([128, F], f32, tag="tp")
                    nc.tensor.matmul(
                        vps[:], ones_sb[:, :128], bv_sb[:], start=True, stop=False
                    )
                    for ec in range(n_ec):
                        nc.tensor.matmul(
                            vps[:],
                            xt[:, ec, sub * 128 : (sub + 1) * 128],
                            wv_sb[:, ec, :],
                            start=False,
                            stop=(ec == n_ec - 1),
                        )
                    tci = tc5 * (IC // 128) + sub
                    for h in range(NH_LOC):
                        nc.vector.tensor_copy(
                            v1_sb[:, tci, h * 66 : h * 66 + 64],
                            vps[:, h * 64 : (h + 1) * 64],
                        )

            if debug_taps:
                nc.sync.dma_start(dbg_qt[:], qt_sb[:])
                nc.sync.dma_start(dbg_kt[:], kt_sb[:])
                nc.sync.dma_start(dbg_v1[:], v1_sb[:])

            # ---- phase C: attention per (batch, local head) ----
            for b in range(n_batch):
                tb = b * tokens_per_batch
                tcb = tb // JC  # first 128-chunk index of this batch
                for h in range(NH_LOC):
                    r0 = h * DK
                    for ic in range(n_ic):
                        qbase = tb + ic * IC
                        njc = (ic + 1) * (IC // JC)
                        pt_tiles = []
                        for jc in range(njc):
                            kbase = tb + jc * JC
                            st = ps_big.tile([128, IC], f32, tag="psb")
                            nc.tensor.matmul(
                                st[:],
                                kt_sb[r0 : r0 + DK, kbase : kbase + JC],
                                qt_sb[r0 : r0 + DK, qbase : qbase + IC],
                                start=True,
                                stop=True,
                            )
                            pt = ptp.tile([128, IC], f32, tag="pt")
                            o = jc - (IC // JC) * ic
                            if o >= 0:
                                tmp = work.tile([128, IC], f32, tag="tmp")
                                nc.vector.tensor_add(tmp[:], st[:], masks_sb[:, o, :])
                                nc.scalar.activation(pt[:], tmp[:], Act.Exp, scale=0.125)
                            else:
                                nc.scalar.activation(pt[:], st[:], Act.Exp, scale=0.125)
                            pt_tiles.append(pt)
                        for sub in range(IC // 128):
                            nj = ic * (IC // JC) + sub + 1
                            yp = ps_y.tile([128, 66], f32, tag="yp")
                            for jj in range(nj):
                                nc.tensor.matmul(
                                    yp[:, 0:65],
                                    pt_tiles[jj][:, sub * 128 : (sub + 1) * 128],
                                    v1_sb[:, tcb + jj, h * 66 : h * 66 + 65],
                                    start=(jj == 0),
                                    stop=(jj == nj - 1),
                                )
                            rd = work.tile([128, 1], f32, tag="rd")
                            nc.vector.reciprocal(rd[:], yp[:, 64:65])
                            qtc = tcb + ic * (IC // 128) + sub
                            nc.scalar.activation(
                                y_sb[:, qtc, r0 : r0 + DK],
                                yp[:, 0:DK],
                                Act.Copy,
                                scale=rd[:],
                            )

            if debug_taps:
                nc.sync.dma_start(dbg_y[:], y_sb[:])

            # ---- phase D: transpose y, output projection ----
            for g in range(n_tc):
                ytp = ps_small.tile([128, 128], f32, tag="tp")
                nc.tensor.transpose(ytp[:], y_sb[:, g, :], ident[:])
                yt = work.tile([128, 128], f32, tag="yt")
                nc.scalar.activation(yt[:], ytp[:], Act.Copy)
                if debug_taps and g == 0:
                    nc.sync.dma_start(dbg_yt[:], yt[:])
                for fc in range(n_fc):
                    ops = ps_big.tile([128, fc_w], f32, tag="psb")
                    nc.tensor.matmul(
                        ops[:],
                        yt[:],
                        wo_sb[:, fc * fc_w : (fc + 1) * fc_w],
                        start=True,
                        stop=True,
                    )
                    ob = outsp.tile([128, fc_w], f32, tag="ob")
                    nc.scalar.activation(ob[:], ops[:], Act.Copy)
                    if debug_taps and g == 0 and fc == 0:
                        nc.sync.dma_start(dbg_ob[:], ob[:])
                    nc.sync.dma_start(
                        out_ap[g * 128 : (g + 1) * 128, fc * fc_w : (fc + 1) * fc_w],
                        ob[:],
                    )

    nc.compile()
    return nc


def _masks():
    j = np.arange(JC)[:, None]
    i = np.arange(IC)[None, :]
    m = np.zeros((4, JC, IC), dtype=np.float32)
    for o in range(4):
        m[o] = np.where(JC * o + j <= i, 0.0, NEG)
    return m


def _get_program():
    if "nc" not in _CACHE:
        _CACHE["nc"] = _build_program(T, B, E)
    return _CACHE["nc"]


def _prepare_in_maps(inputs):
    x = np.ascontiguousarray(np.asarray(inputs["x"], dtype=np.float32).reshape(TT, E))
    Wq = np.asarray(inputs["Wq"], dtype=np.float32)
    Wk = np.asarray(inputs["Wk"], dtype=np.float32)
    Wv = np.asarray(inputs["Wv"], dtype=np.float32)
    Wo = np.asarray(inputs["Wo"], dtype=np.float32)
    bq = np.asarray(inputs["bq"], dtype=np.float32)
    bk = np.asarray(inputs["bk"], dtype=np.float32)
    bv = np.asarray(inputs["bv"], dtype=np.float32)

    masks = _masks()
    ident = np.eye(128, dtype=np.float32)

    in_maps = []
    for c in range(N_CORES):
        sl = slice(c * F, (c + 1) * F)
        in_maps.append(
            {
                "x": x,
                "wqT": np.ascontiguousarray(Wq[sl].T),
                "wkT": np.ascontiguousarray(Wk[sl].T),
                "wvT": np.ascontiguousarray(Wv[sl].T),
                "woT": np.ascontiguousarray(Wo[:, sl].T),
                "bq": np.ascontiguousarray(bq[sl]),
                "bk": np.ascontiguousarray(bk[sl]),
                "bv": np.ascontiguousarray(bv[sl]),
                "masks": masks,
                "ident": ident,
            }
        )
    return in_maps


def kernel(x, Wq, bq, Wk, bk, Wv, bv, Wo, bo):
    from concourse.bass_utils import run_bass_kernel_spmd

    nc = _get_program()
    bo = np.asarray(bo, dtype=np.float32)
    in_maps = _prepare_in_maps(
        {"x": x, "Wq": Wq, "bq": bq, "Wk": Wk, "bk": bk,
         "Wv": Wv, "bv": bv, "Wo": Wo, "bo": bo}
    )

    res = run_bass_kernel_spmd(nc, in_maps, core_ids=list(range(N_CORES)))
    out = np.zeros((TT, E), dtype=np.float64)
    for c in range(N_CORES):
        out += res.results[c]["partial"]
    out += bo[None, :]
    return out.reshape(B, T, E).astype(np.float32)


# revision 24
# speedup vs baseline: 7.4037x; 7.4037x over previous
"""Causal self-attention (B=2, T=2048, E=1024, H=16, d_k=64) on 8 TRN2 cores.

Tensor-parallel over heads: core c owns heads 2c, 2c+1 (feature slice
c*128:(c+1)*128 of the QKV projections and of the Wo contraction dim).
Each core computes a partial output [4096, 1024]; the host sums the 8
partials and adds bo.

All matmuls use float32r (TF32-like, ~1.6e-4 rel err, 4x the fp32 rate
at moving free dim >= 256); accumulation stays fp32 in PSUM.
"""

import numpy as np

B = 2
T = 2048
E = 1024
F = 128          # per-core QKV features (2 heads x 64)
DK = 64
NH_LOC = 2       # heads per core
N_CORES = 8
TT = B * T       # flattened tokens
IC = 512         # query chunk / moving free dim
JC = 128         # key chunk (stationary free dim)
NEG = -1.0e9     # pre-scale mask value; exp(0.125 * NEG) == 0 in fp32

_CACHE = {}


def _build_program(tokens_per_batch, n_batch, embd, debug_taps=False):
    import concourse.mybir as mybir
    import concourse.tile as tile
    from concourse import bacc

    f32 = mybir.dt.float32
    fmm = mybir.dt.float32r          # matmul operand dtype
    Act = mybir.ActivationFunctionType

    t_total = tokens_per_batch * n_batch        # all tokens
    n_ec = embd // 128                          # e-chunks (contraction)
    n_tc5 = t_total // IC                       # 512-token chunks
    n_tc = t_total // JC                        # 128-token chunks
    n_ic = tokens_per_batch // IC               # query chunks per batch
    fc_w = min(IC, embd)                        # output projection chunk width
    n_fc = embd // fc_w                         # output feature chunks

    nc = bacc.Bacc("TRN2", target_bir_lowering=False, debug=False)

    x_ap = nc.dram_tensor("x", [t_total, embd], fmm, kind="ExternalInput").ap()
    wqT = nc.dram_tensor("wqT", [embd, F], fmm, kind="ExternalInput").ap()
    wkT = nc.dram_tensor("wkT", [embd, F], fmm, kind="ExternalInput").ap()
    wvT = nc.dram_tensor("wvT", [embd, F], fmm, kind="ExternalInput").ap()
    woT = nc.dram_tensor("woT", [F, embd], fmm, kind="ExternalInput").ap()
    bq_ap = nc.dram_tensor("bq", [F], f32, kind="ExternalInput").ap()
    bk_ap = nc.dram_tensor("bk", [F], f32, kind="ExternalInput").ap()
    bv_ap = nc.dram_tensor("bv", [F], f32, kind="ExternalInput").ap()
    masks_ap = nc.dram_tensor("masks", [4, JC, IC], f32, kind="ExternalInput").ap()
    ident_ap = nc.dram_tensor("ident", [128, 128], fmm, kind="ExternalInput").ap()
    out_ap = nc.dram_tensor("partial", [t_total, embd], f32, kind="ExternalOutput").ap()
    if debug_taps:
        dbg_qt = nc.dram_tensor("dbg_qt", [128, t_total], f32, kind="ExternalOutput").ap()
        dbg_kt = nc.dram_tensor("dbg_kt", [128, t_total], f32, kind="ExternalOutput").ap()
        dbg_v1 = nc.dram_tensor("dbg_v1", [128, n_tc, 132], f32, kind="ExternalOutput").ap()
        dbg_yt = nc.dram_tensor("dbg_yt", [128, t_total], f32, kind="ExternalOutput").ap()

    with tile.TileContext(nc) as tc:
        with (
            tc.tile_pool(name="const", bufs=1) as constp,
            tc.tile_pool(name="persist", bufs=1) as persist,
            tc.tile_pool(name="xload", bufs=3) as xload,
            tc.tile_pool(name="xt", bufs=2) as xtp,
            tc.tile_pool(name="pt", bufs=6) as ptp,
            tc.tile_pool(name="work", bufs=3) as work,
            tc.tile_pool(name="outs", bufs=3) as outsp,
            tc.tile_pool(name="ps_big", bufs=3, space="PSUM") as ps_big,
            tc.tile_pool(name="ps_small", bufs=2, space="PSUM") as ps_small,
            tc.tile_pool(name="ps_y", bufs=2, space="PSUM") as ps_y,
            tc.tile_pool(name="ps_bc", bufs=1, space="PSUM") as ps_bc,
        ):
            # ---- constants ----
            wq_sb = constp.tile([128, n_ec, F], fmm, tag="wq")
            nc.sync.dma_start(wq_sb[:], wqT.rearrange("(a p) f -> p a f", p=128))
            wk_sb = constp.tile([128, n_ec, F], fmm, tag="wk")
            nc.sync.dma_start(wk_sb[:], wkT.rearrange("(a p) f -> p a f", p=128))
            wv_sb = constp.tile([128, n_ec, F], fmm, tag="wv")
            nc.sync.dma_start(wv_sb[:], wvT.rearrange("(a p) f -> p a f", p=128))
            wo_sb = constp.tile([128, embd], fmm, tag="wo")
            nc.sync.dma_start(wo_sb[:], woT)
            bq_sb = constp.tile([128, 1], f32, tag="bq")
            nc.sync.dma_start(bq_sb[:], bq_ap[:])
            bk_sb = constp.tile([128, 1], f32, tag="bk")
            nc.sync.dma_start(bk_sb[:], bk_ap[:])
            bv_sb = constp.tile([128, 1], f32, tag="bv")
            nc.sync.dma_start(bv_sb[:], bv_ap[:])
            masks_sb = constp.tile([128, 4, IC], f32, tag="masks")
            nc.sync.dma_start(masks_sb[:], masks_ap.rearrange("o p i -> p o i"))
            ident = constp.tile([128, 128], fmm, tag="ident")
            nc.sync.dma_start(ident[:], ident_ap)
            ones_f32 = constp.tile([128, 1], f32, tag="ones_f32")
            nc.vector.memset(ones_f32[:], 1.0)
            ones64 = constp.tile([1, DK], fmm, tag="ones64")
            nc.vector.tensor_copy(
                ones64[:], ones_f32[0:1, 0:1].broadcast_to([1, DK])
            )

            # ---- persistent activations ----
            qt_sb = persist.tile([128, t_total], fmm, tag="qt")   # [f, t]
            kt_sb = persist.tile([128, t_total], fmm, tag="kt")   # [f, t]
            # V plus a ones column per head: head h at cols h*66 .. h*66+64
            v1_sb = persist.tile([128, n_tc, 2 * 66], fmm, tag="v1")
            yt_sb = persist.tile([128, t_total], fmm, tag="ytall")  # [e', t]

            for h in range(NH_LOC):
                nc.vector.tensor_copy(
                    v1_sb[:, :, h * 66 + 64],
                    ones_f32[:, 0:1].broadcast_to([128, n_tc]),
                )
                nc.vector.memset(v1_sb[:, :, h * 66 + 65].bitcast(f32), 0.0)

            # ---- phase B: transpose x, project to QT/KT/VT, V back to [t,f] ----
            for tc5 in range(n_tc5):
                t0 = tc5 * IC
                xt = xtp.tile([128, n_ec, IC], fmm, tag="xt")  # [e, ec, t]
                for sub in range(IC // 128):
                    xrow = xload.tile([128, embd], fmm, tag="xrow")
                    nc.sync.dma_start(
                        xrow[:], x_ap[t0 + sub * 128 : t0 + (sub + 1) * 128, :]
                    )
                    for ec in range(n_ec):
                        tp = ps_small.tile([128, 128], fmm, tag="tp")
                        nc.tensor.transpose(
                            tp[:], xrow[:, ec * 128 : (ec + 1) * 128], ident[:]
                        )
                        nc.vector.tensor_copy(
                            xt[:, ec, sub * 128 : (sub + 1) * 128], tp[:]
                        )
                # QT / KT / VT chunks ([f, t] layout)
                vt = work.tile([128, IC], fmm, tag="vt")
                for dst, w_sb, b_sb in (
                    (qt_sb[:, t0 : t0 + IC], wq_sb, bq_sb),
                    (kt_sb[:, t0 : t0 + IC], wk_sb, bk_sb),
                    (vt[:], wv_sb, bv_sb),
                ):
                    ps = ps_big.tile([128, IC], f32, tag="psb")
                    for ec in range(n_ec):
                        nc.tensor.matmul(
                            ps[:],
                            w_sb[:, ec, :],
                            xt[:, ec, :],
                            start=(ec == 0),
                            stop=(ec == n_ec - 1),
                        )
                    nc.scalar.activation(dst, ps[:], Act.Identity, bias=b_sb[:])
                # V back to natural [t, f] with the ones column
                for sub in range(IC // 128):
                    vp = ps_small.tile([128, 128], fmm, tag="tp")
                    nc.tensor.transpose(
                        vp[:], vt[:, sub * 128 : (sub + 1) * 128], ident[:]
                    )
                    tci = tc5 * (IC // 128) + sub
                    for h in range(NH_LOC):
                        nc.vector.tensor_copy(
                            v1_sb[:, tci, h * 66 : h * 66 + 64],
                            vp[:, h * 64 : (h + 1) * 64],
                        )

            if debug_taps:
                nc.sync.dma_start(dbg_qt[:], qt_sb[:].bitcast(f32))
                nc.sync.dma_start(dbg_kt[:], kt_sb[:].bitcast(f32))
                nc.sync.dma_start(dbg_v1[:], v1_sb[:].bitcast(f32))

            # ---- phase C: attention per (batch, local head) ----
            for b in range(n_batch):
                tb = b * tokens_per_batch
                tcb = tb // JC  # first 128-chunk index of this batch
                for h in range(NH_LOC):
                    r0 = h * DK
                    for ic in range(n_ic):
                        qbase = tb + ic * IC
                        njc = (ic + 1) * (IC // JC)
                        yp = ps_y.tile([65, IC], f32, tag="yp")
                        for jc in range(njc):
                            kbase = tb + jc * JC
                            st = ps_big.tile([128, IC], f32, tag="psb")
                            nc.tensor.matmul(
                                st[:],
                                kt_sb[r0 : r0 + DK, kbase : kbase + JC],
                                qt_sb[r0 : r0 + DK, qbase : qbase + IC],
                                start=True,
                                stop=True,
                            )
                            pt = ptp.tile([128, IC], fmm, tag="pt")
                            o = jc - (IC // JC) * ic
                            if o >= 0:  # diagonal tile: additive causal mask
                                tmp = work.tile([128, IC], f32, tag="tmp")
                                nc.vector.tensor_add(tmp[:], st[:], masks_sb[:, o, :])
                                nc.scalar.activation(pt[:], tmp[:], Act.Exp, scale=0.125)
                            else:
                                nc.scalar.activation(pt[:], st[:], Act.Exp, scale=0.125)
                            nc.tensor.matmul(
                                yp[:],
                                v1_sb[:, tcb + jc, h * 66 : h * 66 + 65],
                                pt[:],
                                start=(jc == 0),
                                stop=(jc == njc - 1),
                            )
                        # normalize: yT rows 0:64 scaled by 1/denominator (row 64)
                        rcp = work.tile([1, IC], f32, tag="rcp")
                        nc.vector.reciprocal(rcp[:], yp[64:65, :])
                        rcr = work.tile([1, IC], fmm, tag="rcr")
                        nc.vector.tensor_copy(rcr[:], rcp[:])
                        bc = ps_bc.tile([DK, IC], f32, tag="bc")
                        nc.tensor.matmul(bc[:], ones64[:], rcr[:], start=True, stop=True)
                        bcs = work.tile([DK, IC], f32, tag="bcs")
                        nc.scalar.activation(bcs[:], bc[:], Act.Copy)
                        nc.vector.tensor_mul(
                            yt_sb[r0 : r0 + DK, qbase : qbase + IC],
                            yp[0:DK, :],
                            bcs[:],
                        )

            if debug_taps:
                nc.sync.dma_start(dbg_yt[:], yt_sb[:].bitcast(f32))

            # ---- phase D: output projection ----
            for g in range(n_tc):
                for fc in range(n_fc):
                    ops = ps_big.tile([128, fc_w], f32, tag="psb")
                    nc.tensor.matmul(
                        ops[:],
                        yt_sb[:, g * 128 : (g + 1) * 128],
                        wo_sb[:, fc * fc_w : (fc + 1) * fc_w],
                        start=True,
                        stop=True,
                    )
                    ob = outsp.tile([128, fc_w], f32, tag="ob")
                    nc.scalar.activation(ob[:], ops[:], Act.Copy)
                    nc.sync.dma_start(
                        out_ap[g * 128 : (g + 1) * 128, fc * fc_w : (fc + 1) * fc_w],
                        ob[:],
                    )

    nc.compile()
    return nc


def _masks():
    j = np.arange(JC)[:, None]
    i = np.arange(IC)[None, :]
    m = np.zeros((4, JC, IC), dtype=np.float32)
    for o in range(4):
        m[o] = np.where(JC * o + j <= i, 0.0, NEG)
    return m


def _get_program():
    if "nc" not in _CACHE:
        _CACHE["nc"] = _build_program(T, B, E)
    return _CACHE["nc"]


def _prepare_in_maps(inputs):
    x = np.ascontiguousarray(np.asarray(inputs["x"], dtype=np.float32).reshape(TT, E))
    Wq = np.asarray(inputs["Wq"], dtype=np.float32)
    Wk = np.asarray(inputs["Wk"], dtype=np.float32)
    Wv = np.asarray(inputs["Wv"], dtype=np.float32)
    Wo = np.asarray(inputs["Wo"], dtype=np.float32)
    bq = np.asarray(inputs["bq"], dtype=np.float32)
    bk = np.asarray(inputs["bk"], dtype=np.float32)
    bv = np.asarray(inputs["bv"], dtype=np.float32)

    masks = _masks()
    ident = np.eye(128, dtype=np.float32)

    in_maps = []
    for c in range(N_CORES):
        sl = slice(c * F, (c + 1) * F)
        in_maps.append(
            {
                "x": x,
                "wqT": np.ascontiguousarray(Wq[sl].T),
                "wkT": np.ascontiguousarray(Wk[sl].T),
                "wvT": np.ascontiguousarray(Wv[sl].T),
                "woT": np.ascontiguousarray(Wo[:, sl].T),
                "bq": np.ascontiguousarray(bq[sl]),
                "bk": np.ascontiguousarray(bk[sl]),
                "bv": np.ascontiguousarray(bv[sl]),
                "masks": masks,
                "ident": ident,
            }
        )
    return in_maps


def kernel(x, Wq, bq, Wk, bk, Wv, bv, Wo, bo):
    from concourse.bass_utils import run_bass_kernel_spmd

    nc = _get_program()
    bo = np.asarray(bo, dtype=np.float32)
    in_maps = _prepare_in_maps(
        {"x": x, "Wq": Wq, "bq": bq, "Wk": Wk, "bk": bk,
         "Wv": Wv, "bv": bv, "Wo": Wo, "bo": bo}
    )

    res = run_bass_kernel_spmd(nc, in_maps, core_ids=list(range(N_CORES)))
    out = np.zeros((TT, E), dtype=np.float64)
    for c in range(N_CORES):
        out += res.results[c]["partial"]
    out += bo[None, :]
    return out.reshape(B, T, E).astype(np.float32)
